# revision 52
# baseline (speedup 1.0000x reference)
"""AdaptiveRotatedConv2d on 8 TRN2 NeuronCores (data-parallel over batch).

Per core (2 samples):
  Stage A: rotated weights rw[b,p,cin,cout] = sum_{n,q} rm[b,n,p,q] * W[n,cout,cin,q]
           via TensorE matmuls: lhsT = rmt [36, 32] (stationary, cols 18..31
           zero), rhs = wf [36, m''] chunks (moving), with wf columns
           host-reordered cout-half-major: m'' = ot*32768 + cin*128 + co.
           DMA tricks: wf tiles alternate between SBUF partition rows 0..35
           and 64..99 (disjoint SDMA engine sets) and between the two HWDGE
           rings (sync/scalar). Outputs packed 4x into PSUM partition
           col-groups via tile_position; one DVE cast per PSUM tile; bounced
           through DRAM scratch [b*9+p, ot, cin, co] (the bounce performs
           the (b,p)->cin transpose).
  Stage B: conv as 9-tap shifted matmuls on the host-W-padded [128, 64, 66]
           image; row-border taps write clipped (contiguous) PSUM
           sub-regions (center tap first, start=True, full coverage; PSUM
           has_written handles partial accumulation). lhsT = rw tiles
           [cin=128, (p, cout=128)]; 18 matmuls per group [128, 8, 64].
  The two stages interleave per cout-half: A(ot=0) -> B(ot=0) -> A(ot=1)
  -> B(ot=1), so stage A(ot=1)'s DMA hides under stage B(ot=0)'s matmuls.
"""

from contextlib import ExitStack

import ml_dtypes
import numpy as np

import concourse.tile as tile
from concourse import bacc, mybir
from concourse.bass_utils import run_bass_kernel_spmd

B, N, COUT, CIN, H, W = 16, 4, 256, 256, 64, 64
NCORES = 8
BPC = B // NCORES            # samples per core
NTAP = 9
KA = N * NTAP                # 36  stage-A contraction (n, q)
MA = BPC * NTAP              # 18  stage-A output rows (b_local, p)
MAP = 32                     # padded to one PE col-strip
M_TOT = CIN * COUT           # 65536
M_HALF = M_TOT // 2          # per cout-half
RPC = 8                      # output rows per PSUM group (N = 8*64 = 512)
NYC = H // RPC               # 8
CT = CIN // 128              # cin tiles (2)
OT = COUT // 128             # cout tiles (2)
WP = W + 2                   # host-padded width

BF16 = mybir.dt.bfloat16
F32 = mybir.dt.float32


def _build_graph():
    nc = bacc.Bacc(None, target_bir_lowering=False)

    x_d = nc.dram_tensor("x", [BPC, CIN, H, WP], BF16, kind="ExternalInput")
    wf_d = nc.dram_tensor("wf", [KA, M_TOT], BF16, kind="ExternalInput")
    rmt_d = nc.dram_tensor("rmt", [128, MAP], BF16, kind="ExternalInput")
    out_d = nc.dram_tensor("out", [BPC, COUT, H, W], F32, kind="ExternalOutput")

    with tile.TileContext(nc) as tc, ExitStack() as ctx:
        const_pool = ctx.enter_context(tc.tile_pool(name="const", bufs=1))
        wf_pool = ctx.enter_context(tc.tile_pool(name="wfin", bufs=4))
        rwst_pool = ctx.enter_context(tc.tile_pool(name="rwst", bufs=2))
        xin_pool = ctx.enter_context(tc.tile_pool(name="xin", bufs=1))
        rwt_pool = ctx.enter_context(tc.tile_pool(name="rwt", bufs=1))
        out_pool = ctx.enter_context(tc.tile_pool(name="outs", bufs=2))
        scr_pool = ctx.enter_context(tc.tile_pool(name="scr", bufs=1, space="DRAM"))
        psa_pool = ctx.enter_context(tc.tile_pool(name="psa", bufs=2, space="PSUM"))
        psb_pool = ctx.enter_context(tc.tile_pool(name="psb", bufs=4, space="PSUM"))

        scr = scr_pool.tile([MA, OT, M_HALF], BF16)

        # rmt rows 0..35 and 64..99 both hold the rotation-mix matrix
        rmt_sb = const_pool.tile([128, MAP], BF16)
        nc.sync.dma_start(rmt_sb[:], rmt_d[:])

        # input images (host-padded width, contiguous loads; issued after
        # each stage-A half's wf loads on the HWDGE rings)
        xs = [[None] * CT for _ in range(BPC)]

        def load_x(b, ct, eng):
            xt = xin_pool.tile([128, H, WP], BF16, tag=f"x{b}{ct}")
            eng.dma_start(xt[:], x_d[b, ct * 128 : (ct + 1) * 128])
            xs[b][ct] = xt

        # chunk c = 16*t + 8*a + 4*k + j; col within half = c*512 + w
        # (j fastest: one matmul wave = 4 col-groups over contiguous 2048 cols)
        scr_w = scr[:].rearrange(
            "m o (t a k j w) -> m o t a k j w", t=4, a=2, k=2, j=4, w=512
        )
        scr_m = scr[:].rearrange("m o (c v) -> m o c v", c=CIN)

        rwts = {}
        TAPS = [(p, p // 3 - 1, p % 3 - 1) for p in [4, 0, 1, 2, 3, 5, 6, 7, 8]]

        def stage_a_half(h):
            # all wf loads of the half first - keeps the HWDGE rings free of
            # head-of-line blocking behind cast-gated scratch writes
            wf_sbs = []
            for t in range(4):
                tg = 4 * h + t
                # h0 is critical path: split across both HWDGE rings.
                # h1 hides under B(ot=0): ride the gpsimd SWDGE queue.
                if h == 0:
                    e_lo, e_hi = nc.sync, nc.scalar
                else:
                    e_lo = e_hi = nc.gpsimd
                wf_sb = wf_pool.tile([128, 16 * 512], BF16, tag="wf")
                nq = 1
                for q in range(nq):
                    sl = 4096 // nq
                    e_lo.dma_start(
                        wf_sb[0:KA, q * sl : (q + 1) * sl],
                        wf_d[:, tg * 8192 + q * sl : tg * 8192 + (q + 1) * sl],
                    )
                    e_hi.dma_start(
                        wf_sb[64 : 64 + KA, 4096 + q * sl : 4096 + (q + 1) * sl],
                        wf_d[:, tg * 8192 + 4096 + q * sl : tg * 8192 + 4096 + (q + 1) * sl],
                    )
                wf_sbs.append(wf_sb)
            rw_sb = None
            for t2 in range(8):
                t = t2 // 2
                wf_sb = wf_sbs[t]
                a = t2 % 2
                row0 = 64 * a
                ps = psa_pool.tile([128, 1024], F32, tag="psa")
                for k in range(2):
                    for j in range(4):
                        c_loc = (a * 8 + 4 * k + j) * 512
                        nc.tensor.matmul(
                            ps[32 * j : 32 * (j + 1), 512 * k : 512 * (k + 1)],
                            rmt_sb[row0 : row0 + KA, :],
                            wf_sb[row0 : row0 + KA, c_loc : c_loc + 512],
                            start=True,
                            stop=True,
                            tile_position=(row0, 32 * j),
                        )
                if a == 0:
                    rw_sb = rwst_pool.tile([128, 2, 2, 512], BF16, tag="rw")
                nc.vector.tensor_copy(
                    rw_sb[:].rearrange("m a k w -> m (a k w)")[
                        :, a * 1024 : (a + 1) * 1024
                    ],
                    ps[:],
                )
                if a == 1:
                    for j in range(4):
                        # split each half's writes across two queues
                        if h == 0:
                            eng = nc.scalar if j in (0, 2) else nc.gpsimd
                        else:
                            eng = nc.gpsimd if j in (0, 2) else nc.scalar
                        eng.dma_start(
                            scr_w[:, h, t, :, :, j, :],
                            rw_sb[32 * j : 32 * j + MA],
                        )
                # rw lhsT tiles: cin half ct only needs wf tiles 2ct..2ct+1
                # of this half, so load right after their bounce writes
                if t2 in (3, 7):
                    ct = t2 // 4
                    for b in range(BPC):
                        rwt = rwt_pool.tile(
                            [128, NTAP, 128], BF16, tag=f"rwt{b}{h}{ct}"
                        )
                        src = scr_m[
                            b * NTAP : (b + 1) * NTAP,
                            h,
                            ct * 128 : (ct + 1) * 128,
                            :,
                        ].rearrange("p c v -> c p v")
                        (nc.scalar if h == 0 else nc.gpsimd).dma_start(rwt[:], src)
                        rwts[(b, h, ct)] = rwt

        def _taps_mm(b, ot, ct, y0, psf, start, stop):
            xt = xs[b][ct]
            for i, (p, dy, dx) in enumerate(TAPS):
                r0 = max(0, -(y0 + dy))
                r1 = RPC - max(0, y0 + RPC - 1 + dy - (H - 1))
                nc.tensor.matmul(
                    psf[:, r0 * W : r1 * W],
                    rwts[(b, ot, ct)][:, p, :],
                    xt[:, y0 + dy + r0 : y0 + dy + r1, dx + 1 : dx + 1 + W],
                    start=(start and i == 0),
                    stop=(stop and i == NTAP - 1),
                )

        def stage_b_block(b, ot, store_eng, split_entry=False, final=False):
            NSPLIT = 2  # yc2 pairs handled via the two-pass entry
            if split_entry:
                # two-pass entry: ct0 taps of the first groups first, so the
                # PE can start before rwt-ct1 / the 2nd x tile arrive
                pss = []
                for yc in range(2 * NSPLIT):
                    ps = psb_pool.tile([128, RPC, W], F32, tag="psb")
                    psf = ps[:].rearrange("m r c -> m (r c)")
                    _taps_mm(b, ot, 0, yc * RPC, psf, True, False)
                    pss.append(ps)
                for yc in range(2 * NSPLIT):
                    psf = pss[yc][:].rearrange("m r c -> m (r c)")
                    _taps_mm(b, ot, 1, yc * RPC, psf, False, True)
            for yc2 in range(NYC // 2):
                if split_entry and yc2 < NSPLIT:
                    ot_sb = out_pool.tile([128, 2 * RPC, W], F32, tag="osb")
                    for half in range(2):
                        nc.vector.tensor_copy(
                            ot_sb[:, half * RPC : (half + 1) * RPC, :],
                            pss[yc2 * 2 + half][:],
                        )
                else:
                    ot_sb = out_pool.tile([128, 2 * RPC, W], F32, tag="osb")
                    for half in range(2):
                        yc = yc2 * 2 + half
                        y0 = yc * RPC
                        ps = psb_pool.tile([128, RPC, W], F32, tag="psb")
                        psf = ps[:].rearrange("m r c -> m (r c)")
                        _taps_mm(b, ot, 0, y0, psf, True, False)
                        _taps_mm(b, ot, 1, y0, psf, False, True)
                        nc.vector.tensor_copy(
                            ot_sb[:, half * RPC : (half + 1) * RPC, :], ps[:]
                        )
                        if final and yc2 == NYC // 2 - 1:
                            store_eng.dma_start(
                                out_d[b, ot * 128 : (ot + 1) * 128,
                                      yc * RPC : (yc + 1) * RPC],
                                ot_sb[:, half * RPC : (half + 1) * RPC, :],
                            )
                if final and yc2 == NYC // 2 - 1:
                    continue
                store_eng.dma_start(
                    out_d[
                        b,
                        ot * 128 : (ot + 1) * 128,
                        yc2 * 2 * RPC : (yc2 + 1) * 2 * RPC,
                    ],
                    ot_sb[:],
                )

        stage_a_half(0)
        load_x(0, 0, nc.sync)
        load_x(0, 1, nc.sync)
        stage_b_block(0, 0, nc.scalar, split_entry=True)
        stage_a_half(1)
        load_x(1, 0, nc.sync)
        load_x(1, 1, nc.sync)
        stage_b_block(1, 0, nc.sync)
        stage_b_block(0, 1, nc.gpsimd)
        stage_b_block(1, 1, nc.sync, final=True)

    nc.compile()
    return nc


_NC = None


def _get_nc():
    global _NC
    if _NC is None:
        _NC = _build_graph()
    return _NC


def _rot_mats_np(thetas):
    """thetas: [M] -> [M, 9, 9], numpy port of the reference builder."""
    thetas = np.asarray(thetas, np.float32)
    xc = np.cos(thetas)
    ys = np.sin(thetas)
    a = xc - ys
    b = xc * ys
    c = xc + ys
    z = np.zeros_like(xc)
    o = np.ones_like(xc)

    def mat(rows):
        return np.stack([np.stack(r, axis=-1) for r in rows], axis=-2)

    pos = mat([
        [a, 1 - a, z, z, z, z, z, z, z],
        [z, xc - b, b, z, 1 - c + b, ys - b, z, z, z],
        [z, z, a, z, z, 1 - a, z, z, z],
        [b, ys - b, z, xc - b, 1 - c + b, z, z, z, z],
        [z, z, z, z, o, z, z, z, z],
        [z, z, z, z, 1 - c + b, xc - b, z, ys - b, b],
        [z, z, z, 1 - a, z, z, a, z, z],
        [z, z, z, ys - b, 1 - c + b, z, b, xc - b, z],
        [z, z, z, z, z, z, z, 1 - a, a],
    ])
    neg = mat([
        [c, z, z, 1 - c, z, z, z, z, z],
        [-b, xc + b, z, b - ys, 1 - a - b, z, z, z, z],
        [z, 1 - c, c, z, z, z, z, z, z],
        [z, z, z, xc + b, 1 - a - b, z, -b, b - ys, z],
        [z, z, z, z, o, z, z, z, z],
        [z, b - ys, -b, z, 1 - a - b, xc + b, z, z, z],
        [z, z, z, z, z, z, c, 1 - c, z],
        [z, z, z, z, 1 - a - b, b - ys, z, xc + b, -b],
        [z, z, z, z, z, 1 - c, z, z, c],
    ])
    m = (thetas >= 0).astype(np.float32)[:, None, None]
    return m * pos + (1 - m) * neg


def _prep_inputs(x, alphas, angles, weight):
    x = np.asarray(x, np.float32)
    alphas = np.asarray(alphas, np.float32)
    angles = np.asarray(angles, np.float32)
    weight = np.asarray(weight, np.float32)

    rm = _rot_mats_np(angles.reshape(-1)).reshape(B, N, NTAP, NTAP)
    rm = rm * alphas[:, :, None, None]          # [b, n, p, q]
    # lhsT rows (n, q) = n*9+q ; cols (b, p) = b*9+p
    rmt = rm.transpose(1, 3, 0, 2).reshape(KA, B, NTAP)

    # wf rows (n, qy, qx); cols m'' = ot*32768 + cin*128 + co
    wf = weight.transpose(0, 3, 4, 2, 1).reshape(KA, CIN, OT, COUT // OT)
    wf = wf.transpose(0, 2, 1, 3).reshape(KA, M_TOT)

    xpad = np.zeros((B, CIN, H, WP), np.float32)
    xpad[:, :, :, 1 : W + 1] = x
    x_bf = xpad.astype(ml_dtypes.bfloat16)
    wf_bf = np.ascontiguousarray(wf).astype(ml_dtypes.bfloat16)

    in_maps = []
    for i in range(NCORES):
        rmt_i = np.zeros((128, MAP), np.float32)
        rmt_i[:KA, :MA] = rmt[:, i * BPC : (i + 1) * BPC].reshape(KA, MA)
        rmt_i[64 : 64 + KA] = rmt_i[:KA]
        in_maps.append({
            "x": np.ascontiguousarray(x_bf[i * BPC : (i + 1) * BPC]),
            "wf": wf_bf,
            "rmt": rmt_i.astype(ml_dtypes.bfloat16),
        })
    return in_maps


def _run(inputs, trace=False, **kw):
    nc = _get_nc()
    in_maps = _prep_inputs(**inputs)
    br = run_bass_kernel_spmd(nc, in_maps, core_ids=list(range(NCORES)),
                              trace=trace, **kw)
    out = np.concatenate([r["out"] for r in br.results], axis=0)
    return out, br


def kernel(x, alphas, angles, weight):
    out, _ = _run(dict(x=x, alphas=alphas, angles=angles, weight=weight))
    return out


if __name__ == "__main__":
    rng = np.random.default_rng(0)
    x = rng.standard_normal((B, CIN, H, W), np.float32)
    alphas = rng.random((B, N), np.float32)
    angles = (rng.standard_normal((B, N), np.float32) * 0.5).astype(np.float32)
    weight = rng.standard_normal((N, COUT, CIN, 3, 3), np.float32) * np.sqrt(
        2.0 / (COUT * 9)
    ).astype(np.float32)
    out = kernel(x=x, alphas=alphas, angles=angles, weight=weight)
    print(out.shape, out.dtype, np.abs(out).mean())


# revision 55
# speedup vs baseline: 1.2073x; 1.2073x over previous
"""AdaptiveRotatedConv2d on 8 TRN2 NeuronCores (data-parallel over batch).

Per core (2 samples):
  Stage A: rotated weights rw[b,p,cin,cout] = sum_{n,q} rm[b,n,p,q] * W[n,cout,cin,q]
           via TensorE matmuls: lhsT = rmt [36, 32] (stationary, cols 18..31
           zero), rhs = wf [36, m''] chunks (moving), with wf columns
           host-reordered cout-half-major: m'' = ot*32768 + cin*128 + co.
           DMA tricks: wf tiles alternate between SBUF partition rows 0..35
           and 64..99 (disjoint SDMA engine sets) and between the two HWDGE
           rings (sync/scalar). Outputs packed 4x into PSUM partition
           col-groups via tile_position; one DVE cast per PSUM tile; bounced
           through DRAM scratch [b*9+p, ot, cin, co] (the bounce performs
           the (b,p)->cin transpose).
  Stage B: conv as 9-tap shifted matmuls on the host-W-padded [128, 64, 66]
           image; row-border taps write clipped (contiguous) PSUM
           sub-regions (center tap first, start=True, full coverage; PSUM
           has_written handles partial accumulation). lhsT = rw tiles
           [cin=128, (p, cout=128)]; 18 matmuls per group [128, 8, 64].
  The two stages interleave per cout-half: A(ot=0) -> B(ot=0) -> A(ot=1)
  -> B(ot=1), so stage A(ot=1)'s DMA hides under stage B(ot=0)'s matmuls.
"""

from contextlib import ExitStack

import ml_dtypes
import numpy as np

import concourse.tile as tile
from concourse import bacc, mybir
from concourse.bass_utils import run_bass_kernel_spmd

B, N, COUT, CIN, H, W = 16, 4, 256, 256, 64, 64
NCORES = 8
BPC = B // NCORES            # samples per core
NTAP = 9
KA = N * NTAP                # 36  stage-A contraction (n, q)
MA = BPC * NTAP              # 18  stage-A output rows (b_local, p)
MAP = 32                     # padded to one PE col-strip
M_TOT = CIN * COUT           # 65536
M_HALF = M_TOT // 2          # per cout-half
RPC = 8                      # output rows per PSUM group (N = 8*64 = 512)
NYC = H // RPC               # 8
CT = CIN // 128              # cin tiles (2)
OT = COUT // 128             # cout tiles (2)
WP = W + 2                   # host-padded width

BF16 = mybir.dt.bfloat16
F32 = mybir.dt.float32


def _build_graph():
    nc = bacc.Bacc(None, target_bir_lowering=False)

    x_d = nc.dram_tensor("x", [BPC, CIN, H, WP], BF16, kind="ExternalInput")
    wf_d = nc.dram_tensor("wf", [KA, M_TOT], BF16, kind="ExternalInput")
    rmt_d = nc.dram_tensor("rmt", [128, MAP], BF16, kind="ExternalInput")
    out_d = nc.dram_tensor("out", [BPC, COUT, H, W], F32, kind="ExternalOutput")

    with tile.TileContext(nc) as tc, ExitStack() as ctx:
        const_pool = ctx.enter_context(tc.tile_pool(name="const", bufs=1))
        wf_pool = ctx.enter_context(tc.tile_pool(name="wfin", bufs=4))
        rwst_pool = ctx.enter_context(tc.tile_pool(name="rwst", bufs=2))
        xin_pool = ctx.enter_context(tc.tile_pool(name="xin", bufs=1))
        rwt_pool = ctx.enter_context(tc.tile_pool(name="rwt", bufs=1))
        out_pool = ctx.enter_context(tc.tile_pool(name="outs", bufs=2))
        scr_pool = ctx.enter_context(tc.tile_pool(name="scr", bufs=1, space="DRAM"))
        psa_pool = ctx.enter_context(tc.tile_pool(name="psa", bufs=2, space="PSUM"))
        psb_pool = ctx.enter_context(tc.tile_pool(name="psb", bufs=4, space="PSUM"))

        scr = scr_pool.tile([MA, OT, M_HALF], BF16)

        # rmt rows 0..35 and 64..99 both hold the rotation-mix matrix
        rmt_sb = const_pool.tile([128, MAP], BF16)
        nc.sync.dma_start(rmt_sb[:], rmt_d[:])

        # input images (host-padded width, contiguous loads; issued after
        # each stage-A half's wf loads on the HWDGE rings)
        xs = [[None] * CT for _ in range(BPC)]

        def load_x(b, ct, eng):
            xt = xin_pool.tile([128, H, WP], BF16, tag=f"x{b}{ct}")
            eng.dma_start(xt[:], x_d[b, ct * 128 : (ct + 1) * 128])
            xs[b][ct] = xt

        # chunk c = 16*t + 8*a + 2*j + k; col within half = c*512 + w
        scr_w = scr[:].rearrange(
            "m o (t a j k w) -> m o t a j k w", t=4, a=2, j=4, k=2, w=512
        )
        scr_m = scr[:].rearrange("m o (c v) -> m o c v", c=CIN)

        rwts = {}
        TAPS = [(p, p // 3 - 1, p % 3 - 1) for p in [4, 0, 1, 2, 3, 5, 6, 7, 8]]

        def stage_a_half(h):
            # all wf loads of the half first - keeps the HWDGE rings free of
            # head-of-line blocking behind cast-gated scratch writes
            wf_sbs = []
            for t in range(4):
                tg = 4 * h + t
                # h0 is critical path: split across both HWDGE rings.
                # h1 hides under B(ot=0): ride the gpsimd SWDGE queue.
                if h == 0:
                    e_lo, e_hi = nc.sync, nc.scalar
                else:
                    e_lo = e_hi = nc.gpsimd
                wf_sb = wf_pool.tile([128, 16 * 512], BF16, tag="wf")
                nq = 1
                for q in range(nq):
                    sl = 4096 // nq
                    e_lo.dma_start(
                        wf_sb[0:KA, q * sl : (q + 1) * sl],
                        wf_d[:, tg * 8192 + q * sl : tg * 8192 + (q + 1) * sl],
                    )
                    e_hi.dma_start(
                        wf_sb[64 : 64 + KA, 4096 + q * sl : 4096 + (q + 1) * sl],
                        wf_d[:, tg * 8192 + 4096 + q * sl : tg * 8192 + 4096 + (q + 1) * sl],
                    )
                wf_sbs.append(wf_sb)
            rw_sb = None
            for t2 in range(8):
                t = t2 // 2
                wf_sb = wf_sbs[t]
                a = t2 % 2
                row0 = 64 * a
                ps = psa_pool.tile([128, 1024], F32, tag="psa")
                for j in range(4):
                    for k in range(2):
                        c_loc = (a * 8 + 2 * j + k) * 512
                        nc.tensor.matmul(
                            ps[32 * j : 32 * (j + 1), 512 * k : 512 * (k + 1)],
                            rmt_sb[row0 : row0 + KA, :],
                            wf_sb[row0 : row0 + KA, c_loc : c_loc + 512],
                            start=True,
                            stop=True,
                            tile_position=(row0, 32 * j),
                        )
                if a == 0:
                    rw_sb = rwst_pool.tile([128, 2, 2, 512], BF16, tag="rw")
                nc.vector.tensor_copy(
                    rw_sb[:].rearrange("m a k w -> m (a k w)")[
                        :, a * 1024 : (a + 1) * 1024
                    ],
                    ps[:],
                )
                if a == 1:
                    for j in range(4):
                        # split each half's writes across two queues
                        if h == 0:
                            eng = nc.scalar if j in (0, 2) else nc.gpsimd
                        else:
                            eng = nc.gpsimd if j in (0, 2) else nc.scalar
                        eng.dma_start(
                            scr_w[:, h, t, :, j, :, :],
                            rw_sb[32 * j : 32 * j + MA],
                        )
                # rw lhsT tiles: cin half ct only needs wf tiles 2ct..2ct+1
                # of this half, so load right after their bounce writes
                if t2 in (3, 7):
                    ct = t2 // 4
                    for b in range(BPC):
                        rwt = rwt_pool.tile(
                            [128, NTAP, 128], BF16, tag=f"rwt{b}{h}{ct}"
                        )
                        src = scr_m[
                            b * NTAP : (b + 1) * NTAP,
                            h,
                            ct * 128 : (ct + 1) * 128,
                            :,
                        ].rearrange("p c v -> c p v")
                        (nc.scalar if h == 0 else nc.gpsimd).dma_start(rwt[:], src)
                        rwts[(b, h, ct)] = rwt

        def _taps_mm(b, ot, ct, y0, psf, start, stop):
            xt = xs[b][ct]
            for i, (p, dy, dx) in enumerate(TAPS):
                r0 = max(0, -(y0 + dy))
                r1 = RPC - max(0, y0 + RPC - 1 + dy - (H - 1))
                nc.tensor.matmul(
                    psf[:, r0 * W : r1 * W],
                    rwts[(b, ot, ct)][:, p, :],
                    xt[:, y0 + dy + r0 : y0 + dy + r1, dx + 1 : dx + 1 + W],
                    start=(start and i == 0),
                    stop=(stop and i == NTAP - 1),
                )

        def stage_b_block(b, ot, store_eng, split_entry=False, final=False):
            NSPLIT = 2  # yc2 pairs handled via the two-pass entry
            if split_entry:
                # two-pass entry: ct0 taps of the first groups first, so the
                # PE can start before rwt-ct1 / the 2nd x tile arrive
                pss = []
                for yc in range(2 * NSPLIT):
                    ps = psb_pool.tile([128, RPC, W], F32, tag="psb")
                    psf = ps[:].rearrange("m r c -> m (r c)")
                    _taps_mm(b, ot, 0, yc * RPC, psf, True, False)
                    pss.append(ps)
                for yc in range(2 * NSPLIT):
                    psf = pss[yc][:].rearrange("m r c -> m (r c)")
                    _taps_mm(b, ot, 1, yc * RPC, psf, False, True)
            for yc2 in range(NYC // 2):
                if split_entry and yc2 < NSPLIT:
                    ot_sb = out_pool.tile([128, 2 * RPC, W], F32, tag="osb")
                    for half in range(2):
                        nc.vector.tensor_copy(
                            ot_sb[:, half * RPC : (half + 1) * RPC, :],
                            pss[yc2 * 2 + half][:],
                        )
                else:
                    ot_sb = out_pool.tile([128, 2 * RPC, W], F32, tag="osb")
                    for half in range(2):
                        yc = yc2 * 2 + half
                        y0 = yc * RPC
                        ps = psb_pool.tile([128, RPC, W], F32, tag="psb")
                        psf = ps[:].rearrange("m r c -> m (r c)")
                        _taps_mm(b, ot, 0, y0, psf, True, False)
                        _taps_mm(b, ot, 1, y0, psf, False, True)
                        nc.vector.tensor_copy(
                            ot_sb[:, half * RPC : (half + 1) * RPC, :], ps[:]
                        )
                        if final and yc2 == NYC // 2 - 1:
                            store_eng.dma_start(
                                out_d[b, ot * 128 : (ot + 1) * 128,
                                      yc * RPC : (yc + 1) * RPC],
                                ot_sb[:, half * RPC : (half + 1) * RPC, :],
                            )
                if final and yc2 == NYC // 2 - 1:
                    continue
                store_eng.dma_start(
                    out_d[
                        b,
                        ot * 128 : (ot + 1) * 128,
                        yc2 * 2 * RPC : (yc2 + 1) * 2 * RPC,
                    ],
                    ot_sb[:],
                )

        stage_a_half(0)
        load_x(0, 0, nc.sync)
        load_x(0, 1, nc.sync)
        stage_b_block(0, 0, nc.scalar, split_entry=True)
        stage_a_half(1)
        load_x(1, 0, nc.sync)
        load_x(1, 1, nc.sync)
        stage_b_block(1, 0, nc.sync)
        stage_b_block(0, 1, nc.gpsimd)
        stage_b_block(1, 1, nc.sync, final=True)

    nc.compile()
    return nc


_NC = None


def _get_nc():
    global _NC
    if _NC is None:
        _NC = _build_graph()
    return _NC


def _rot_mats_np(thetas):
    """thetas: [M] -> [M, 9, 9], numpy port of the reference builder."""
    thetas = np.asarray(thetas, np.float32)
    xc = np.cos(thetas)
    ys = np.sin(thetas)
    a = xc - ys
    b = xc * ys
    c = xc + ys
    z = np.zeros_like(xc)
    o = np.ones_like(xc)

    def mat(rows):
        return np.stack([np.stack(r, axis=-1) for r in rows], axis=-2)

    pos = mat([
        [a, 1 - a, z, z, z, z, z, z, z],
        [z, xc - b, b, z, 1 - c + b, ys - b, z, z, z],
        [z, z, a, z, z, 1 - a, z, z, z],
        [b, ys - b, z, xc - b, 1 - c + b, z, z, z, z],
        [z, z, z, z, o, z, z, z, z],
        [z, z, z, z, 1 - c + b, xc - b, z, ys - b, b],
        [z, z, z, 1 - a, z, z, a, z, z],
        [z, z, z, ys - b, 1 - c + b, z, b, xc - b, z],
        [z, z, z, z, z, z, z, 1 - a, a],
    ])
    neg = mat([
        [c, z, z, 1 - c, z, z, z, z, z],
        [-b, xc + b, z, b - ys, 1 - a - b, z, z, z, z],
        [z, 1 - c, c, z, z, z, z, z, z],
        [z, z, z, xc + b, 1 - a - b, z, -b, b - ys, z],
        [z, z, z, z, o, z, z, z, z],
        [z, b - ys, -b, z, 1 - a - b, xc + b, z, z, z],
        [z, z, z, z, z, z, c, 1 - c, z],
        [z, z, z, z, 1 - a - b, b - ys, z, xc + b, -b],
        [z, z, z, z, z, 1 - c, z, z, c],
    ])
    m = (thetas >= 0).astype(np.float32)[:, None, None]
    return m * pos + (1 - m) * neg


def _prep_inputs(x, alphas, angles, weight):
    x = np.asarray(x, np.float32)
    alphas = np.asarray(alphas, np.float32)
    angles = np.asarray(angles, np.float32)
    weight = np.asarray(weight, np.float32)

    rm = _rot_mats_np(angles.reshape(-1)).reshape(B, N, NTAP, NTAP)
    rm = rm * alphas[:, :, None, None]          # [b, n, p, q]
    # lhsT rows (n, q) = n*9+q ; cols (b, p) = b*9+p
    rmt = rm.transpose(1, 3, 0, 2).reshape(KA, B, NTAP)

    # wf rows (n, qy, qx); cols m'' = ot*32768 + cin*128 + co
    wf = weight.transpose(0, 3, 4, 2, 1).reshape(KA, CIN, OT, COUT // OT)
    wf = wf.transpose(0, 2, 1, 3).reshape(KA, M_TOT)

    xpad = np.zeros((B, CIN, H, WP), np.float32)
    xpad[:, :, :, 1 : W + 1] = x
    x_bf = xpad.astype(ml_dtypes.bfloat16)
    wf_bf = np.ascontiguousarray(wf).astype(ml_dtypes.bfloat16)

    in_maps = []
    for i in range(NCORES):
        rmt_i = np.zeros((128, MAP), np.float32)
        rmt_i[:KA, :MA] = rmt[:, i * BPC : (i + 1) * BPC].reshape(KA, MA)
        rmt_i[64 : 64 + KA] = rmt_i[:KA]
        in_maps.append({
            "x": np.ascontiguousarray(x_bf[i * BPC : (i + 1) * BPC]),
            "wf": wf_bf,
            "rmt": rmt_i.astype(ml_dtypes.bfloat16),
        })
    return in_maps


def _run(inputs, trace=False, **kw):
    nc = _get_nc()
    in_maps = _prep_inputs(**inputs)
    br = run_bass_kernel_spmd(nc, in_maps, core_ids=list(range(NCORES)),
                              trace=trace, **kw)
    out = np.concatenate([r["out"] for r in br.results], axis=0)
    return out, br


def kernel(x, alphas, angles, weight):
    out, _ = _run(dict(x=x, alphas=alphas, angles=angles, weight=weight))
    return out


if __name__ == "__main__":
    rng = np.random.default_rng(0)
    x = rng.standard_normal((B, CIN, H, W), np.float32)
    alphas = rng.random((B, N), np.float32)
    angles = (rng.standard_normal((B, N), np.float32) * 0.5).astype(np.float32)
    weight = rng.standard_normal((N, COUT, CIN, 3, 3), np.float32) * np.sqrt(
        2.0 / (COUT * 9)
    ).astype(np.float32)
    out = kernel(x=x, alphas=alphas, angles=angles, weight=weight)
    print(out.shape, out.dtype, np.abs(out).mean())
